# revision 3
# baseline (speedup 1.0000x reference)
"""CrossFormer forward pass on 8 Trainium2 NeuronCores.

Strategy (see spec sharding_hint): the input-dominated front end (CubeEmbedding
conv3d + channel-LN + CEL0) is sharded 8-way over image rows with halo — each
core reads only its 1/8 slice of the 439MB input (rows [80r-28, 80r+108)) and
produces its 10-row band of the stage-0 feature map; bands are all-gathered.
The remaining stages (window attention blocks on <=3.3MB tensors) run on
device as a single compiled program. Self-contained; hardcoded shapes.
"""
import numpy as np
import jax
import jax.numpy as jnp
from jax import lax
from jax.sharding import Mesh, PartitionSpec as P
from jax.experimental.shard_map import shard_map
from functools import partial

IMG_H, IMG_W = 640, 1280
FRAMES, PATCH = 2, 2
IN_CH = 67
DIMS = (64, 128, 256, 512)
DEPTHS = (2, 2, 8, 2)
GWSZ = (10, 5, 2, 1)
LWSZ = (10, 10, 10, 10)
CEL_KS = ((4, 8, 16, 32), (2, 4), (2, 4), (2, 4))
CEL_ST = (4, 2, 2, 2)
DIM_HEAD = 32
EPS = 1e-5
NC = 8
HALO = 28          # input rows of halo each side (28 = 2*14 cube-out halo)
XROWS = 136        # 80 owned + 2*28 halo


def _conv2d(x, w, b=None, stride=1, pad=0):
    y = lax.conv_general_dilated(x, w, (stride, stride), [(pad, pad), (pad, pad)],
                                 dimension_numbers=('NCHW', 'OIHW', 'NCHW'))
    return y if b is None else y + b[None, :, None, None]


def _ln_last(x, g, b):
    mu = x.mean(-1, keepdims=True)
    var = jnp.var(x, axis=-1, keepdims=True)
    return (x - mu) / jnp.sqrt(var + EPS) * g + b


def _chan_ln(x, g, b):
    mu = x.mean(1, keepdims=True)
    var = jnp.var(x, axis=1, keepdims=True)
    return (x - mu) / jnp.sqrt(var + EPS) * g + b


def _group_norm(x, groups, w, b):
    B, C, H, W = x.shape
    xr = x.reshape(B, groups, C // groups, H, W)
    mu = xr.mean((2, 3, 4), keepdims=True)
    var = jnp.var(xr, axis=(2, 3, 4), keepdims=True)
    xr = (xr - mu) / jnp.sqrt(var + EPS)
    return xr.reshape(B, C, H, W) * w[None, :, None, None] + b[None, :, None, None]


def _dpb_mlp(p, r):
    h = jax.nn.relu(_ln_last(r @ p['w1'].T + p['b1'], p['g1'], p['e1']))
    h = jax.nn.relu(_ln_last(h @ p['w2'].T + p['b2'], p['g2'], p['e2']))
    h = jax.nn.relu(_ln_last(h @ p['w3'].T + p['b3'], p['g3'], p['e3']))
    return (h @ p['w4'].T + p['b4'])[:, 0]


def _rel_pos_tables(wsz):
    pos = np.arange(wsz)
    grid = np.stack(np.meshgrid(pos, pos, indexing='ij')).reshape(2, -1).T
    rel = grid[:, None] - grid[None, :] + (wsz - 1)
    idx = (rel * np.array([2 * wsz - 1, 1])).sum(-1)
    p2 = np.arange(-wsz, wsz + 1)
    mg = np.stack(np.meshgrid(p2, p2, indexing='ij')).reshape(2, -1).T.astype(np.float32)
    return jnp.asarray(idx), jnp.asarray(mg)


def _attention(x, p, attn_type, wsz):
    B, D, H, W = x.shape
    heads = D // DIM_HEAD
    nb = B * (H // wsz) * (W // wsz)
    xn = _chan_ln(x, p['ng'], p['nb'])
    if attn_type == 'short':
        xw = xn.reshape(B, D, H // wsz, wsz, W // wsz, wsz).transpose(0, 2, 4, 1, 3, 5)
    else:
        xw = xn.reshape(B, D, wsz, H // wsz, wsz, W // wsz).transpose(0, 3, 5, 1, 2, 4)
    xw = xw.reshape(nb, D, wsz, wsz)
    qkv = _conv2d(xw, p['qkv']).reshape(nb, 3, heads, DIM_HEAD, wsz * wsz)
    q, k, v = qkv[:, 0], qkv[:, 1], qkv[:, 2]
    sim = jnp.einsum('bhdi,bhdj->bhij', q * DIM_HEAD ** -0.5, k)
    idx, mg = _rel_pos_tables(wsz)
    bias = _dpb_mlp(p['dpb'], mg)[idx]
    attnw = jax.nn.softmax(sim + bias, axis=-1)
    out = jnp.einsum('bhij,bhdj->bhdi', attnw, v).reshape(nb, D, wsz, wsz)
    out = _conv2d(out, p['ow'], p['ob'])
    out = out.reshape(B, H // wsz, W // wsz, D, wsz, wsz)
    if attn_type == 'short':
        out = out.transpose(0, 3, 1, 4, 2, 5)
    else:
        out = out.transpose(0, 3, 4, 1, 5, 2)
    return out.reshape(B, D, H, W)


def _feed_forward(x, p):
    y = _chan_ln(x, p['ng'], p['nb'])
    y = jax.nn.gelu(_conv2d(y, p['w1'], p['b1']), approximate=False)
    return _conv2d(y, p['w2'], p['b2'])


# ---------------- phase A: sharded cube + LN + CEL0 ----------------

def _phase_a_shard(xs, cube, cel):
    """xs: (67, 2, 136, 1280) one core's input rows (zero-padded at edges).
    Returns this core's y0 band (64, 10, 160)."""
    cid = lax.axis_index('c')
    # cube conv: VALID, stride (2,2,2) -> (64, 1, 68, 640)
    y = lax.conv_general_dilated(xs[None], cube['w'], (2, 2, 2), 'VALID',
                                 dimension_numbers=('NCDHW', 'OIDHW', 'NCDHW'))
    y = (y + cube['b'][None, :, None, None, None])[0, :, 0]       # (64, 68, 640)
    # channel LN per pixel
    mu = y.mean(0, keepdims=True)
    var = jnp.var(y, axis=0, keepdims=True)
    y = (y - mu) / jnp.sqrt(var + EPS) * cube['lnw'][:, None, None] + cube['lnb'][:, None, None]
    # zero rows outside the global image (cube rows g = 40*cid - 14 + l)
    l = jnp.arange(68)
    g = 40 * cid - 14 + l
    y = y * ((g >= 0) & (g < 320))[None, :, None].astype(y.dtype)
    # CEL0: stride 4; H handled via halo (VALID on row windows), W zero-padded
    outs = []
    row0 = {4: 14, 8: 12, 16: 8, 32: 0}
    for (wk, bk), kk in zip(cel, (4, 8, 16, 32)):
        pad = (kk - 4) // 2
        r0 = row0[kk]
        rows = r0 + 0
        o = lax.conv_general_dilated(
            y[None][:, :, rows:rows + 36 + kk, :], wk, (4, 4),
            [(0, 0), (pad, pad)], dimension_numbers=('NCHW', 'OIHW', 'NCHW'))
        outs.append(o[0] + bk[:, None, None])
    y0 = jnp.concatenate(outs, 0)                                  # (64, 10, 160)
    return y0


def _cel(y, cel_params, si):
    kernels = sorted(CEL_KS[si])
    st = CEL_ST[si]
    fmaps = [_conv2d(y, cp['w'], cp['b'], stride=st, pad=(k - st) // 2)
             for cp, k in zip(cel_params, kernels)]
    return jnp.concatenate(fmaps, axis=1)


def _block(y, blk, si):
    y = _attention(y, blk['sa'], 'short', LWSZ[si]) + y
    y = _feed_forward(y, blk['sf']) + y
    y = _attention(y, blk['la'], 'long', GWSZ[si]) + y
    y = _feed_forward(y, blk['lf']) + y
    return y


def _up_fc(y, u, f):
    B, C, H, W = y.shape
    O = u['ctw'].shape[1]
    up = jnp.einsum('bcij,copq->boipjq', y, u['ctw']).reshape(B, O, 2 * H, 2 * W)
    up = up + u['ctb'][None, :, None, None]
    z = jax.nn.silu(_group_norm(_conv2d(up, u['c1w'], u['c1b'], pad=1), DIMS[0], u['g1w'], u['g1b']))
    z = jax.nn.silu(_group_norm(_conv2d(z, u['c2w'], u['c2b'], pad=1), DIMS[0], u['g2w'], u['g2b']))
    y = z + up
    return y.transpose(0, 2, 3, 1) @ f['w'].T + f['b']


_COMPILED = {}


def _get_fns():
    if _COMPILED:
        return _COMPILED
    devs = jax.devices()[:NC]
    mesh = Mesh(np.asarray(devs), ('c',))

    def pa(xsh, cube, cel):
        y0 = _phase_a_shard(xsh[0], cube, cel)
        y0g = lax.all_gather(y0, 'c')                 # (8, 64, 10, 160)
        return y0g.transpose(1, 0, 2, 3).reshape(64, 80, 160)[None]

    phase_a = jax.jit(shard_map(
        pa, mesh=mesh,
        in_specs=(P('c'), P(), P()), out_specs=P(),
        check_rep=False))

    # per-stage jits: one modest compile per distinct graph (bounded compile
    # time vs one monolithic full-model graph), reused across same-stage blocks
    _COMPILED['pa'] = phase_a
    _COMPILED['cel'] = {si: jax.jit(partial(_cel, si=si)) for si in (1, 2, 3)}
    _COMPILED['blk'] = {si: jax.jit(partial(_block, si=si)) for si in range(4)}
    _COMPILED['upfc'] = jax.jit(_up_fc)
    _COMPILED['mesh'] = mesh
    return _COMPILED


def kernel(x, params):
    x = np.asarray(x)
    fns = _get_fns()
    # host-side slice: core r gets input rows [80r-28, 80r+108), zero-padded
    xs = np.zeros((NC, IN_CH, FRAMES, XROWS, IMG_W), np.float32)
    for r in range(NC):
        lo, hi = 80 * r - HALO, 80 * r + 80 + HALO
        clo, chi = max(lo, 0), min(hi, IMG_H)
        xs[r, :, :, clo - lo:clo - lo + (chi - clo), :] = x[0, :, :, clo:chi, :]
    y = fns['pa'](jnp.asarray(xs), params['cube'],
                  tuple((p['w'], p['b']) for p in params['stages'][0]['cel']))
    for si in range(4):
        if si > 0:
            y = fns['cel'][si](y, params['stages'][si]['cel'])
        for blk in params['stages'][si]['blocks']:
            y = fns['blk'][si](y, blk)
    out = fns['upfc'](y, params['up'], params['fc'])
    return np.asarray(out)
